# revision 22
# baseline (speedup 1.0000x reference)
"""Expected Calibration Error kernel for 8 Trainium2 NeuronCores.

Strategy (data-parallel over rows, per the sharding hint):
  - Host prep: pad N=1,000,000 rows to 1,007,616 = 8 * 123 * 1024 with
    all-zero logit rows; gather the label logit g[r] = logits[r, labels[r]]
    (pure input indexing prep) so the device checks accuracy with one exact
    f32 compare acc = (g == max) instead of an argmax.
  - Per batch of 8 tiles [128p x (8 rows * 100 cls)], single-pass engines:
      e = exp(logits)                     ACT, unshifted (|l| < 7)
      m = segmented max(logits)           DVE tensor_reduce axis=X (batched)
      S = segmented sum(e):               Pool pairwise folds 100->50->25
                                          (one batched inst per fold level),
                                          DVE reduce over 25 finishes
        (schedule knob: 'P1' folds once, DVE reduces 50; 'D' all-DVE)
  - Epilogue, chunked (16 tiles = 128 staged cols), FIFO-deferred so it
    overlaps the main loop:
      em = exp(m); sinv = 1/S             ACT / DVE
      conf = em*sinv, acc = (g == m)      DVE TT, written strided into the
                                          matmul lhsT staging [ones|conf|acc]
      binv = (15*conf - .5) + MAGIC       two fused ACT Copy (scale+bias)
      one-hot oh[., col, b] = (binv == MAGIC+b)
                                          15 tensor_scalar is_equal insts per
                                          chunk (DVE 2x f32 mode; some bins
                                          optionally on Pool via knob)
      per 32-col window: one PE matmul psum[65,480] += lhsT[128,65]^T @ oh
        row 0 = counts, rows 1+j = conf sums, rows 33+j = acc sums (col-slot
        diagonal j), accumulated over all windows in one PSUM bank.
  - Host: extract (count, conf_sum, acc_sum)[15] from the [65,480] dumps,
    sum over cores/slots, subtract the pad rows' contribution (bin 0, acc 0
    since g_pad = -1 != m_pad = 0), finish the ECE reduction.
"""

import sys

sys.path.insert(0, "/opt/trn_rl_repo")

import numpy as np

import concourse.hw_specs as hw_specs
import concourse.bass as bass
import concourse.mybir as mybir
import concourse.tile as tile
from concourse.vector_clock import ScopedClock

# Calibrate the Tile scheduler's cost model to rates measured on this HW
# (repeat-slope micro-benchmarks): Pool TensorTensor runs ~2.48 ns/elem (Q7
# software op), not the 0.833 the default model assumes, and ACT ~0.75.
# With realistic durations the static schedule stops serializing around
# underestimated Pool slots.  Must run before the first cost-model use.
hw_specs.TRN2Spec.CYCLE_T[mybir.EngineType.Pool] = 1.89
hw_specs.TRN2Spec.CYCLE_T[mybir.EngineType.Activation] = 0.75

F32 = mybir.dt.float32

N_BINS = 15
C = 100
ROWS_PER_TILE = 1024
A = 8  # rows per partition per tile
N_FULL = 1_000_000
N_CORES = 8
TILES = 123
ROWS_PER_CORE = TILES * ROWS_PER_TILE  # 125952
N_PAD = N_CORES * ROWS_PER_CORE       # 1007616
WIN = 32            # staged cols per matmul window
LHSW = 2 * WIN + 1  # lhsT cols per window: [ones | conf*32 | acc*32] = 65
RHSW = WIN * N_BINS  # rhs cols per window = 480
N_WINS = (TILES * A + WIN - 1) // WIN  # 31; staged cols padded to 31*32 = 992
CHUNK_TILES = 32    # epilogue chunk granularity (32 tiles = 256 staged cols)
MAGIC = 12582912.0  # 1.5*2^23: adding it rounds f32 to the nearest integer


def _patch_tile_drain():
    """walrus in this container allows only 1 sync wait per instruction; split
    every multi-wait instruction's extra waits onto preceding same-engine
    no-ops, and the TileContext exit drain's waits across a chain of drains."""
    if getattr(tile.TileContext, "_drain_patched", False):
        return

    orig_lower = tile.TileContext._lower_ordered_insts

    def _lower_ordered_insts(self, ordered):
        for insts in ordered.values():
            if not isinstance(insts, list):
                continue
            new = []
            for inst in insts:
                si = getattr(inst, "sync_info", None)
                waits = list(si.on_wait) if si is not None else []
                if len(waits) > 1 and isinstance(inst, mybir.Instruction):
                    si.on_wait = waits[-1:]
                    for j, w in enumerate(waits[:-1]):
                        nop = mybir.InstNoOp(
                            name=f"{inst.name}-ws{j}",
                            sync_info=mybir.SyncInfo(on_wait=[w], on_update=[]),
                            bass_nofuse=True,
                            engine=inst.engine,
                        )
                        new.append(nop)
                new.append(inst)
            insts[:] = new
        return orig_lower(self, ordered)

    tile.TileContext._lower_ordered_insts = _lower_ordered_insts

    orig_commit = tile.TileContext._commit_instruction

    def _commit_instruction(self, inst, *args, **kwargs):
        si = getattr(inst, "sync_info", None)
        if si is not None and len(si.on_wait) > 1 and isinstance(inst, mybir.Instruction):
            waits = list(si.on_wait)
            si.on_wait = waits[-1:]
            for j, w in enumerate(waits[:-1]):
                nop = mybir.InstNoOp(
                    name=f"{inst.name}-cs{j}",
                    sync_info=mybir.SyncInfo(on_wait=[w], on_update=[]),
                    bass_nofuse=True,
                    engine=inst.engine,
                )
                orig_commit(self, nop)
        return orig_commit(self, inst, *args, **kwargs)

    tile.TileContext._commit_instruction = _commit_instruction

    def _drain_and_barrier(self, tick_clock, wait_clock):
        drain_inst = self.nc.sync.drain()
        wait_clock.add_sem_waits(
            drain_inst.ins, ScopedClock({None: tick_clock.global_clock})
        )
        si = drain_inst.ins.sync_info
        waits = list(si.on_wait) if si else []
        if len(waits) > 1:
            si.on_wait = waits[:1]
            for i in range(1, len(waits)):
                d2 = self.nc.sync.drain()
                d2.ins.sync_info = type(si)(on_wait=waits[i : i + 1], on_update=[])
        self.nc.all_engine_barrier()
        popped = self.nc._tile_sem_poison_stack.pop()
        assert popped is self._sem_poison
        self.nc.clear_and_free_semaphores(list(self.sems.allocated().values()))
        self.nc.all_engine_barrier()

    tile.TileContext._drain_and_barrier = _drain_and_barrier
    tile.TileContext._drain_patched = True


def build_nc(tiles=TILES, repeat=1, schedule=None, oh_dve_bins=0, ablate="",
             dma_batch=8, io_bufs=2, ek_bufs=2, fifo_depth=4, bodies=1,
             debug_out=False):
    """schedule: per-BATCH sum modes ('P2'|'P1'|'D'): P2 = Pool folds
    100->50->25 + DVE reduce 25; P1 = Pool fold 100->50 + DVE reduce 50;
    D = DVE reduce 100 straight from e.  Default: balanced P2/P1 mix.
    oh_dve_bins: bins [0, k) built on DVE (tensor_scalar is_equal, 2x mode),
    bins [k, 15) on ACT (Square + Relu indicator on the exact integer bin).
    ablate: letters to skip ops for timing diagnostics:
            'm' max, 'e' exp, 's' sum, 'E' epilogue."""
    _patch_tile_drain()
    cols = tiles * A  # staged per-row columns (real)
    n_wins = (cols + WIN - 1) // WIN
    cols_pad = n_wins * WIN  # padded so every matmul window is full
    n_batches = (tiles + dma_batch - 1) // dma_batch
    if schedule is None:
        # Pool folds measured ~1.89 ns/elem: deep folds everywhere keeps both
        # Pool (~142us) and DVE (~136us) under the ~152us DMA floor
        schedule = ["P2"] * n_batches
    assert len(schedule) == n_batches
    assert repeat % bodies == 0 or repeat == 1, (repeat, bodies)

    nc = bass.Bass(trn_type="TRN2")
    lg = nc.declare_dram_parameter("lg", [tiles * ROWS_PER_TILE, C], F32, isOutput=False)
    gg = nc.declare_dram_parameter("gg", [128, cols], F32, isOutput=False)
    part = nc.declare_dram_parameter("part", [LHSW, RHSW], F32, isOutput=True)
    dbg = None
    if debug_out:
        dbg = nc.declare_dram_parameter("dbg", [128, cols * 4], F32, isOutput=True)

    # epilogue chunks; keep the FINAL chunk tiny (last ragged DMA batch) so
    # the serial post-last-DMA tail (exp->fold->finish->epilogue->matmul of
    # whatever loads last) is as short as possible
    last_cs = (tiles - 1) // dma_batch * dma_batch  # first tile of last batch
    chunk_starts = [cs for cs in range(0, tiles, CHUNK_TILES) if cs < last_cs]
    chunk_starts.append(last_cs)
    chunk_bounds = list(zip(chunk_starts, chunk_starts[1:] + [tiles]))

    with tile.TileContext(nc) as tc:
        with (
            tc.tile_pool(name="io", bufs=io_bufs) as io_pool,
            tc.tile_pool(name="ek", bufs=ek_bufs) as ek_pool,
            tc.tile_pool(name="f1", bufs=2) as f1_pool,
            tc.tile_pool(name="f2", bufs=3) as f2_pool,
            tc.tile_pool(name="epi", bufs=2) as epi_pool,
            tc.tile_pool(name="stage", bufs=1) as stage,
            tc.tile_pool(name="psum", bufs=1, space="PSUM") as psum_pool,
        ):
            # ---- persistent staging ----
            m_all = stage.tile([128, cols_pad], F32, tag="m_all")
            s_all = stage.tile([128, cols_pad], F32, tag="s_all")
            g_all = stage.tile([128, cols_pad], F32, tag="g_all")
            # per-bin bias constants for the ACT one-hot: negb[:, b] = -b
            negb_i = stage.tile([128, N_BINS], mybir.dt.int32, tag="negb_i")
            nc.gpsimd.iota(negb_i[:], pattern=[[1, N_BINS]], base=0,
                           channel_multiplier=0)
            negb = stage.tile([128, N_BINS], F32, tag="negb")
            nc.vector.tensor_copy(negb[:], negb_i[:])
            nc.vector.tensor_scalar(negb[:], negb[:], -1.0, None,
                                    op0=mybir.AluOpType.mult)
            one_c = stage.tile([128, 1], F32, tag="one_c")
            nc.vector.memset(one_c[:], 1.0)
            # matmul lhsT staging: n_wins windows of [ones | conf32 | acc32]
            lhsT_all = stage.tile([128, n_wins * LHSW], F32, tag="lhsT_all")
            lhsT_w = lhsT_all[:].rearrange("p (w k) -> p w k", k=LHSW)
            nc.vector.memset(lhsT_w[:, :, 0:1], 1.0)  # ones column per window
            if cols_pad > cols:
                # pad cols: conf = exp(1)/0.5 = 5.44 -> binv = MAGIC+81 ->
                # one-hot all zero -> no histogram contribution
                nc.vector.memset(m_all[:, cols:cols_pad], 1.0)
                nc.vector.memset(s_all[:, cols:cols_pad], 0.5)
                nc.vector.memset(g_all[:, cols:cols_pad], 0.0)

            nc.sync.dma_start(out=g_all[:, :cols], in_=gg[:, :])

            psum = psum_pool.tile([LHSW, RHSW], F32, tag="psum")

            # ---- main loop (optionally repeated on-device for timing) ----
            import contextlib

            n_iters = repeat // bodies if repeat > 1 else 1
            n_bodies = bodies if repeat > 1 else 1
            loop_cm = tc.For_i(0, n_iters, 1) if n_iters > 1 else contextlib.nullcontext()
            with loop_cm:
                # deferred-emission FIFO: consumers of freshly produced data
                # are emitted >= 1 batch (~9 us) after their producers so no
                # in-order engine queue ever stalls on a cross-engine dep.
                pending = []

                def drain(k):
                    for _ in range(min(k, len(pending))):
                        pending.pop(0)()

                for bi in range(n_bodies * n_batches):
                    bi = bi % n_batches
                    drain(max(0, len(pending) - fifo_depth))
                    t0 = bi * dma_batch
                    nb = min(dma_batch, tiles - t0)  # ragged last batch
                    mode = schedule[bi]
                    l_batch = io_pool.tile([128, dma_batch * A * C], F32, tag="l_batch")
                    r0 = t0 * ROWS_PER_TILE
                    nc.sync.dma_start(
                        out=l_batch[:, : nb * A * C].rearrange(
                            "p (b x) -> p b x", x=A * C
                        ),
                        in_=lg[r0 : r0 + nb * ROWS_PER_TILE, :].rearrange(
                            "(b p a) c -> p b a c", p=128, a=A
                        ),
                    )
                    lb = l_batch[:, : nb * A * C]
                    l4 = lb.rearrange("p (b a c) -> p b a c", a=A, c=C)

                    if "e" not in ablate:
                        e_batch = ek_pool.tile([128, dma_batch * A * C], F32, tag="e_batch")
                        eb = e_batch[:, : nb * A * C]
                        nc.scalar.activation(eb, lb, mybir.ActivationFunctionType.Exp)

                    m_slice = m_all[:, t0 * A : (t0 + nb) * A]
                    s_slice = s_all[:, t0 * A : (t0 + nb) * A]

                    if "m" not in ablate:
                        nc.vector.tensor_reduce(
                            out=m_slice, in_=l4, axis=mybir.AxisListType.X,
                            op=mybir.AluOpType.max,
                        )

                    if "s" not in ablate and "e" not in ablate:
                        e4 = eb.rearrange("p (b a c) -> p b a c", a=A, c=C)
                        if mode == "D":
                            def finish(e4=e4, s_slice=s_slice):
                                nc.vector.tensor_reduce(
                                    out=s_slice, in_=e4, axis=mybir.AxisListType.X,
                                    op=mybir.AluOpType.add,
                                )
                            pending.append(finish)
                        elif mode == "P1":
                            h1 = f1_pool.tile([128, dma_batch * A * 50], F32, tag="h1")
                            h14 = h1[:, : nb * A * 50].rearrange(
                                "p (b a c) -> p b a c", a=A, c=50
                            )
                            nc.gpsimd.tensor_tensor(
                                out=h14, in0=e4[:, :, :, 0:50], in1=e4[:, :, :, 50:100],
                                op=mybir.AluOpType.add,
                            )

                            def finish(h14=h14, s_slice=s_slice):
                                nc.vector.tensor_reduce(
                                    out=s_slice, in_=h14, axis=mybir.AxisListType.X,
                                    op=mybir.AluOpType.add,
                                )
                            pending.append(finish)
                        else:  # P2
                            h1 = f1_pool.tile([128, dma_batch * A * 50], F32, tag="h1")
                            h14 = h1[:, : nb * A * 50].rearrange(
                                "p (b a c) -> p b a c", a=A, c=50
                            )
                            nc.gpsimd.tensor_tensor(
                                out=h14, in0=e4[:, :, :, 0:50], in1=e4[:, :, :, 50:100],
                                op=mybir.AluOpType.add,
                            )
                            h2 = f2_pool.tile([128, dma_batch * A * 25], F32, tag="h2")
                            h24 = h2[:, : nb * A * 25].rearrange(
                                "p (b a c) -> p b a c", a=A, c=25
                            )
                            nc.gpsimd.tensor_tensor(
                                out=h24, in0=h14[:, :, :, 0:25], in1=h14[:, :, :, 25:50],
                                op=mybir.AluOpType.add,
                            )

                            def finish(h24=h24, s_slice=s_slice):
                                nc.vector.tensor_reduce(
                                    out=s_slice, in_=h24, axis=mybir.AxisListType.X,
                                    op=mybir.AluOpType.add,
                                )
                            pending.append(finish)

                    # ---- chunked epilogue, staged through the FIFO ----
                    if "E" not in ablate:
                        for cs, ce in chunk_bounds:
                            last = ce - 1
                            if not (t0 <= last < t0 + nb):
                                continue
                            # window-aligned col range (last chunk takes the
                            # pad cols so every window is full)
                            c0 = cs * A
                            c1 = -(-(ce * A) // WIN) * WIN  # round up to window
                            w = c1 - c0
                            w0 = c0 // WIN                     # first window
                            cw = w // WIN                      # windows in chunk
                            assert cw * WIN == w and c0 % WIN == 0
                            # tiny final chunk: all-DVE one-hot (short serial
                            # tail beats ACT's per-instruction overhead there)
                            k_chunk = N_BINS if (cs, ce) == chunk_bounds[-1] else oh_dve_bins

                            box = {}

                            def stage_a(c0=c0, c1=c1, w=w, box=box):
                                em = epi_pool.tile([128, w], F32, tag="em", name="em")
                                nc.scalar.activation(
                                    em[:], m_all[:, c0:c1], mybir.ActivationFunctionType.Exp
                                )
                                sinv = epi_pool.tile([128, w], F32, tag="sinv", name="sinv")
                                nc.vector.reciprocal(sinv[:], s_all[:, c0:c1])
                                box["a"] = (em, sinv)

                            def stage_b(c0=c0, c1=c1, w0=w0, cw=cw, box=box):
                                em, sinv = box["a"]
                                # conf/acc written strided into the lhsT windows
                                conf_dst = lhsT_w[:, w0 : w0 + cw, 1 : 1 + WIN]
                                acc_dst = lhsT_w[:, w0 : w0 + cw, 1 + WIN : LHSW]
                                nc.vector.tensor_tensor(
                                    out=conf_dst,
                                    in0=em[:].rearrange("p (w k) -> p w k", k=WIN),
                                    in1=sinv[:].rearrange("p (w k) -> p w k", k=WIN),
                                    op=mybir.AluOpType.mult,
                                )
                                nc.vector.tensor_tensor(
                                    out=acc_dst,
                                    in0=g_all[:, c0:c1].rearrange("p (w k) -> p w k", k=WIN),
                                    in1=m_all[:, c0:c1].rearrange("p (w k) -> p w k", k=WIN),
                                    op=mybir.AluOpType.is_equal,
                                )
                                box["conf"] = conf_dst

                            def stage_c(w=w, box=box):
                                conf_dst = box["conf"]
                                # ks = exact integer bin index as small f32:
                                # round(15*conf - .5) via the MAGIC add/sub
                                t = epi_pool.tile([128, w], F32, tag="bt", name="bt")
                                nc.scalar.activation(
                                    t[:].rearrange("p (w k) -> p w k", k=WIN),
                                    conf_dst,
                                    mybir.ActivationFunctionType.Copy,
                                    bias=-0.5, scale=float(N_BINS),
                                )
                                binv = epi_pool.tile([128, w], F32, tag="binv", name="binv")
                                nc.scalar.activation(
                                    binv[:], t[:], mybir.ActivationFunctionType.Copy,
                                    bias=MAGIC, scale=1.0,
                                )
                                ks = epi_pool.tile([128, w], F32, tag="ks", name="ks")
                                nc.scalar.activation(
                                    ks[:], binv[:], mybir.ActivationFunctionType.Copy,
                                    bias=-MAGIC, scale=1.0,
                                )
                                box["ks"] = ks

                            def stage_d(w=w, k_chunk=k_chunk, box=box):
                                ks = box["ks"]
                                oh = epi_pool.tile([128, w * N_BINS], F32, tag="oh", name="oh")
                                # bin-major: each bin's one-hot write is a
                                # CONTIGUOUS [128, w] run (strided writes
                                # measured 2-4x slower on ACT/DVE)
                                oh3 = oh[:].rearrange("p (b c) -> p b c", c=w)
                                u = epi_pool.tile([128, w], F32, tag="u", name="u")
                                for b in range(N_BINS):
                                    if b < k_chunk:
                                        nc.vector.tensor_scalar(
                                            oh3[:, b], ks[:], float(b), None,
                                            op0=mybir.AluOpType.is_equal,
                                        )
                                    else:
                                        # u = (ks - b)^2 ; oh_b = relu(1 - u)
                                        # exact {0,1}: ks, b are small ints
                                        nc.scalar.activation(
                                            u[:], ks[:],
                                            mybir.ActivationFunctionType.Square,
                                            bias=negb[:, b : b + 1], scale=1.0,
                                        )
                                        nc.scalar.activation(
                                            oh3[:, b], u[:],
                                            mybir.ActivationFunctionType.Relu,
                                            bias=one_c[:, 0:1], scale=-1.0,
                                        )
                                box["oh"] = oh

                            def stage_e(cs=cs, w=w, w0=w0, cw=cw, box=box):
                                oh = box["oh"]
                                oh3 = oh[:].rearrange("p (b c) -> p b c", c=w)
                                for wi in range(cw):
                                    gw = w0 + wi
                                    first = cs == 0 and wi == 0
                                    final = cs == chunk_starts[-1] and wi == cw - 1
                                    lhsT = lhsT_all[:, gw * LHSW : (gw + 1) * LHSW]
                                    rhs = oh3[:, :, wi * WIN : (wi + 1) * WIN]
                                    nc.tensor.matmul(
                                        out=psum[:, :],
                                        lhsT=lhsT,
                                        rhs=rhs,
                                        start=first,
                                        stop=final,
                                    )

                            pending.extend([stage_a, stage_b, stage_c, stage_d, stage_e])

                drain(len(pending))

            if "E" not in ablate:
                out_sb = stage.tile([LHSW, RHSW], F32, tag="out_sb")
                nc.vector.tensor_copy(out_sb[:], psum[:])
                nc.sync.dma_start(out=part[:], in_=out_sb[:])
            if dbg is not None:
                for i, src in enumerate((m_all, s_all, g_all)):
                    nc.sync.dma_start(out=dbg[:, i * cols : (i + 1) * cols], in_=src[:])

    return nc


# ----------------------------------------------------------------------------
# host side
# ----------------------------------------------------------------------------

_RUNNER_CACHE = {}


def _get_runner(tiles=TILES, repeat=1, ncores=N_CORES, **opts):
    """Build (once) a jitted shard_map runner for the kernel."""
    key = (tiles, repeat, ncores, tuple(sorted((k, str(v)) for k, v in opts.items())))
    if key in _RUNNER_CACHE:
        return _RUNNER_CACHE[key]

    import jax
    from jax.sharding import Mesh, PartitionSpec
    try:
        from jax.experimental.shard_map import shard_map
    except ImportError:
        from jax.shard_map import shard_map
    from concourse import bass2jax

    nc = build_nc(tiles, repeat=repeat, **opts)
    bass2jax.install_neuronx_cc_hook()

    partition_name = nc.partition_id_tensor.name if nc.partition_id_tensor else None
    in_names = ["lg", "gg"]
    out_names = ["part"]
    out_avals = [jax.core.ShapedArray((LHSW, RHSW), np.float32)]
    if opts.get("debug_out"):
        out_names.append("dbg")
        out_avals.append(jax.core.ShapedArray((128, tiles * A * 4), np.float32))
    all_in_names = in_names + out_names + ([partition_name] if partition_name else [])

    def _body(*args):
        operands = list(args)
        if partition_name is not None:
            operands.append(bass2jax.partition_id_tensor())
        outs = bass2jax._bass_exec_p.bind(
            *operands,
            out_avals=tuple(out_avals),
            in_names=tuple(all_in_names),
            out_names=tuple(out_names),
            lowering_input_output_aliases=(),
            sim_require_finite=True,
            sim_require_nnan=True,
            nc=nc,
        )
        return tuple(outs)

    devices = jax.devices()[:ncores]
    mesh = Mesh(np.asarray(devices), ("core",))
    n_in = len(in_names) + len(out_avals)
    sharded = jax.jit(
        shard_map(
            _body,
            mesh=mesh,
            in_specs=(PartitionSpec("core"),) * n_in,
            out_specs=(PartitionSpec("core"),) * len(out_names),
            check_rep=False,
        ),
        donate_argnums=(len(in_names),),
        keep_unused=True,
    )
    _RUNNER_CACHE[key] = sharded
    return sharded


def _prep_inputs(logits, labels):
    logits = np.asarray(logits)
    labels = np.asarray(labels).astype(np.int64)
    n = logits.shape[0]
    assert logits.shape == (N_FULL, C) and n == N_FULL, logits.shape
    pad = N_PAD - n
    lg = np.concatenate([logits, np.zeros((pad, C), np.float32)], axis=0)
    # label-logit gather; pad rows get -1 (pad m = 0 -> acc = 0, bin 0)
    g = np.empty(N_PAD, np.float32)
    g[:n] = logits[np.arange(n), labels]
    g[n:] = -1.0
    # arrange to the device layout: gg[core][p, t*A + a] = g[core, t, p, a]
    gg = np.ascontiguousarray(
        g.reshape(N_CORES, TILES, 128, A).transpose(0, 2, 1, 3)
    ).reshape(N_CORES * 128, TILES * A)
    return lg, gg, pad


def _finish(parts, pad):
    """parts: (8, 65, 480) psum dumps -> ECE scalar (f32 [1]).

    psum col j*15+b holds, for window col-slot j: count in row 0, conf_sum
    in row 1+j, acc_sum in row 33+j (other rows unused cross terms)."""
    v = parts.reshape(parts.shape[0], LHSW, N_BINS, WIN).astype(np.float64)
    j = np.arange(WIN)
    counts = v[:, 0, :, :].sum(axis=(0, 2))
    conf_sums = v[:, 1 + j, :, j].sum(axis=(0, 1))
    acc_sums = v[:, 1 + WIN + j, :, j].sum(axis=(0, 1))
    # all-zero pad rows: conf = exp(0)/100 -> bin 0, acc = 0 (g=-1 != m=0)
    counts[0] -= pad
    conf_sums[0] -= pad * float(np.float32(1.0) / np.float32(100.0))
    n = N_FULL
    prop = counts / n
    denom = np.maximum(counts, 1.0)
    avg_conf = conf_sums / denom
    avg_acc = acc_sums / denom
    per_bin = np.where(counts > 0, np.abs(avg_conf - avg_acc) * prop, 0.0)
    return np.array([per_bin.sum()], dtype=np.float32)


def kernel(logits, labels):
    lg, gg, pad = _prep_inputs(logits, labels)
    runner = _get_runner()
    zeros = np.zeros((N_CORES * LHSW, RHSW), np.float32)
    last = None
    for attempt in range(3):
        try:
            (out,) = runner(lg, gg, np.zeros_like(zeros))
            parts = np.asarray(out).reshape(N_CORES, LHSW, RHSW)
            return _finish(parts, pad)
        except Exception as e:  # transient NRT_EXEC_UNIT_UNRECOVERABLE etc.
            last = e
            import time as _time

            _time.sleep(20)
    raise last


# revision 25
# speedup vs baseline: 1.2400x; 1.2400x over previous
"""Expected Calibration Error kernel for 8 Trainium2 NeuronCores.

Strategy (data-parallel over rows, per the sharding hint):
  - Host prep: pad N=1,000,000 rows to 1,007,616 = 8 * 123 * 1024 with
    all-zero logit rows; gather the label logit g[r] = logits[r, labels[r]]
    (pure input indexing prep) so the device checks accuracy with one exact
    f32 compare acc = (g == max) instead of an argmax.
  - Per batch of 8 tiles [128p x (8 rows * 100 cls)], single-pass engines:
      e = exp(logits)                     ACT, unshifted (|l| < 7)
      m = segmented max(logits)           DVE tensor_reduce axis=X (batched)
      S = segmented sum(e):               Pool pairwise folds 100->50->25
                                          (one batched inst per fold level),
                                          DVE reduce over 25 finishes
        (schedule knob: 'P1' folds once, DVE reduces 50; 'D' all-DVE)
  - Epilogue, chunked (16 tiles = 128 staged cols), FIFO-deferred so it
    overlaps the main loop:
      em = exp(m); sinv = 1/S             ACT / DVE
      conf = em*sinv, acc = (g == m)      DVE TT, written strided into the
                                          matmul lhsT staging [ones|conf|acc]
      binv = (15*conf - .5) + MAGIC       two fused ACT Copy (scale+bias)
      one-hot oh[., col, b] = (binv == MAGIC+b)
                                          15 tensor_scalar is_equal insts per
                                          chunk (DVE 2x f32 mode; some bins
                                          optionally on Pool via knob)
      per 32-col window: one PE matmul psum[65,480] += lhsT[128,65]^T @ oh
        row 0 = counts, rows 1+j = conf sums, rows 33+j = acc sums (col-slot
        diagonal j), accumulated over all windows in one PSUM bank.
  - Host: extract (count, conf_sum, acc_sum)[15] from the [65,480] dumps,
    sum over cores/slots, subtract the pad rows' contribution (bin 0, acc 0
    since g_pad = -1 != m_pad = 0), finish the ECE reduction.
"""

import sys

sys.path.insert(0, "/opt/trn_rl_repo")

import numpy as np

import concourse.hw_specs as hw_specs
import concourse.bass as bass
import concourse.mybir as mybir
import concourse.tile as tile
from concourse.vector_clock import ScopedClock

# Calibrate the Tile scheduler's cost model to rates measured on this HW
# (repeat-slope micro-benchmarks): Pool TensorTensor runs ~2.48 ns/elem (Q7
# software op), not the 0.833 the default model assumes, and ACT ~0.75.
# With realistic durations the static schedule stops serializing around
# underestimated Pool slots.  Must run before the first cost-model use.
hw_specs.TRN2Spec.CYCLE_T[mybir.EngineType.Pool] = 1.89
hw_specs.TRN2Spec.CYCLE_T[mybir.EngineType.Activation] = 0.75

F32 = mybir.dt.float32

N_BINS = 15
C = 100
ROWS_PER_TILE = 1024
A = 8  # rows per partition per tile
N_FULL = 1_000_000
N_CORES = 8
TILES = 123
ROWS_PER_CORE = TILES * ROWS_PER_TILE  # 125952
N_PAD = N_CORES * ROWS_PER_CORE       # 1007616
WIN = 32            # staged cols per matmul window
LHSW = 2 * WIN + 1  # lhsT cols per window: [ones | conf*32 | acc*32] = 65
RHSW = WIN * N_BINS  # rhs cols per window = 480
N_WINS = (TILES * A + WIN - 1) // WIN  # 31; staged cols padded to 31*32 = 992
CHUNK_TILES = 32    # epilogue chunk granularity (32 tiles = 256 staged cols)
MAGIC = 12582912.0  # 1.5*2^23: adding it rounds f32 to the nearest integer


def _patch_tile_drain():
    """walrus in this container allows only 1 sync wait per instruction; split
    every multi-wait instruction's extra waits onto preceding same-engine
    no-ops, and the TileContext exit drain's waits across a chain of drains."""
    if getattr(tile.TileContext, "_drain_patched", False):
        return

    orig_lower = tile.TileContext._lower_ordered_insts

    def _lower_ordered_insts(self, ordered):
        for insts in ordered.values():
            if not isinstance(insts, list):
                continue
            new = []
            for inst in insts:
                si = getattr(inst, "sync_info", None)
                waits = list(si.on_wait) if si is not None else []
                if len(waits) > 1 and isinstance(inst, mybir.Instruction):
                    si.on_wait = waits[-1:]
                    for j, w in enumerate(waits[:-1]):
                        nop = mybir.InstNoOp(
                            name=f"{inst.name}-ws{j}",
                            sync_info=mybir.SyncInfo(on_wait=[w], on_update=[]),
                            bass_nofuse=True,
                            engine=inst.engine,
                        )
                        new.append(nop)
                new.append(inst)
            insts[:] = new
        return orig_lower(self, ordered)

    tile.TileContext._lower_ordered_insts = _lower_ordered_insts

    orig_commit = tile.TileContext._commit_instruction

    def _commit_instruction(self, inst, *args, **kwargs):
        si = getattr(inst, "sync_info", None)
        if si is not None and len(si.on_wait) > 1 and isinstance(inst, mybir.Instruction):
            waits = list(si.on_wait)
            si.on_wait = waits[-1:]
            for j, w in enumerate(waits[:-1]):
                nop = mybir.InstNoOp(
                    name=f"{inst.name}-cs{j}",
                    sync_info=mybir.SyncInfo(on_wait=[w], on_update=[]),
                    bass_nofuse=True,
                    engine=inst.engine,
                )
                orig_commit(self, nop)
        return orig_commit(self, inst, *args, **kwargs)

    tile.TileContext._commit_instruction = _commit_instruction

    def _drain_and_barrier(self, tick_clock, wait_clock):
        drain_inst = self.nc.sync.drain()
        wait_clock.add_sem_waits(
            drain_inst.ins, ScopedClock({None: tick_clock.global_clock})
        )
        si = drain_inst.ins.sync_info
        waits = list(si.on_wait) if si else []
        if len(waits) > 1:
            si.on_wait = waits[:1]
            for i in range(1, len(waits)):
                d2 = self.nc.sync.drain()
                d2.ins.sync_info = type(si)(on_wait=waits[i : i + 1], on_update=[])
        self.nc.all_engine_barrier()
        popped = self.nc._tile_sem_poison_stack.pop()
        assert popped is self._sem_poison
        self.nc.clear_and_free_semaphores(list(self.sems.allocated().values()))
        self.nc.all_engine_barrier()

    tile.TileContext._drain_and_barrier = _drain_and_barrier
    tile.TileContext._drain_patched = True


def build_nc(tiles=TILES, repeat=1, schedule=None, oh_dve_bins=2, ablate="",
             dma_batch=6, io_bufs=3, ek_bufs=2, fifo_depth=4, bodies=1,
             oh_binmajor=True, debug_out=False):
    """schedule: per-BATCH sum modes ('P2'|'P1'|'D'): P2 = Pool folds
    100->50->25 + DVE reduce 25; P1 = Pool fold 100->50 + DVE reduce 50;
    D = DVE reduce 100 straight from e.  Default: balanced P2/P1 mix.
    oh_dve_bins: bins [0, k) built on DVE (tensor_scalar is_equal, 2x mode),
    bins [k, 15) on ACT (Square + Relu indicator on the exact integer bin).
    ablate: letters to skip ops for timing diagnostics:
            'm' max, 'e' exp, 's' sum, 'E' epilogue."""
    _patch_tile_drain()
    cols = tiles * A  # staged per-row columns (real)
    n_wins = (cols + WIN - 1) // WIN
    cols_pad = n_wins * WIN  # padded so every matmul window is full
    n_batches = (tiles + dma_batch - 1) // dma_batch
    if schedule is None:
        # Pool folds measured ~1.89 ns/elem: deep folds everywhere keeps both
        # Pool (~142us) and DVE (~136us) under the ~152us DMA floor
        schedule = ["P2"] * n_batches
    assert len(schedule) == n_batches
    assert repeat % bodies == 0 or repeat == 1, (repeat, bodies)

    nc = bass.Bass(trn_type="TRN2")
    lg = nc.declare_dram_parameter("lg", [tiles * ROWS_PER_TILE, C], F32, isOutput=False)
    gg = nc.declare_dram_parameter("gg", [128, cols], F32, isOutput=False)
    part = nc.declare_dram_parameter("part", [LHSW, RHSW], F32, isOutput=True)
    dbg = None
    if debug_out:
        dbg = nc.declare_dram_parameter("dbg", [128, cols * 4], F32, isOutput=True)

    # epilogue chunks; keep the FINAL chunk tiny (last ragged DMA batch) so
    # the serial post-last-DMA tail (exp->fold->finish->epilogue->matmul of
    # whatever loads last) is as short as possible
    last_cs = (tiles - 1) // dma_batch * dma_batch  # first tile of last batch
    chunk_starts = [cs for cs in range(0, tiles, CHUNK_TILES) if cs < last_cs]
    chunk_starts.append(last_cs)
    chunk_bounds = list(zip(chunk_starts, chunk_starts[1:] + [tiles]))

    with tile.TileContext(nc) as tc:
        with (
            tc.tile_pool(name="io", bufs=io_bufs) as io_pool,
            tc.tile_pool(name="ek", bufs=ek_bufs) as ek_pool,
            tc.tile_pool(name="f1", bufs=2) as f1_pool,
            tc.tile_pool(name="f2", bufs=3) as f2_pool,
            tc.tile_pool(name="epi", bufs=2) as epi_pool,
            tc.tile_pool(name="stage", bufs=1) as stage,
            tc.tile_pool(name="psum", bufs=1, space="PSUM") as psum_pool,
        ):
            # ---- persistent staging ----
            m_all = stage.tile([128, cols_pad], F32, tag="m_all")
            s_all = stage.tile([128, cols_pad], F32, tag="s_all")
            g_all = stage.tile([128, cols_pad], F32, tag="g_all")
            # per-bin bias constants for the ACT one-hot: negb[:, b] = -b
            negb_i = stage.tile([128, N_BINS], mybir.dt.int32, tag="negb_i")
            nc.gpsimd.iota(negb_i[:], pattern=[[1, N_BINS]], base=0,
                           channel_multiplier=0)
            negb = stage.tile([128, N_BINS], F32, tag="negb")
            nc.vector.tensor_copy(negb[:], negb_i[:])
            nc.vector.tensor_scalar(negb[:], negb[:], -1.0, -MAGIC,
                                    op0=mybir.AluOpType.mult,
                                    op1=mybir.AluOpType.add)
            one_c = stage.tile([128, 1], F32, tag="one_c")
            nc.vector.memset(one_c[:], 1.0)
            # matmul lhsT staging: n_wins windows of [ones | conf32 | acc32]
            lhsT_all = stage.tile([128, n_wins * LHSW], F32, tag="lhsT_all")
            lhsT_w = lhsT_all[:].rearrange("p (w k) -> p w k", k=LHSW)
            nc.vector.memset(lhsT_w[:, :, 0:1], 1.0)  # ones column per window
            if cols_pad > cols:
                # pad cols: conf = exp(1)/0.5 = 5.44 -> binv = MAGIC+81 ->
                # one-hot all zero -> no histogram contribution
                nc.vector.memset(m_all[:, cols:cols_pad], 1.0)
                nc.vector.memset(s_all[:, cols:cols_pad], 0.5)
                nc.vector.memset(g_all[:, cols:cols_pad], 0.0)

            nc.sync.dma_start(out=g_all[:, :cols], in_=gg[:, :])

            psum = psum_pool.tile([LHSW, RHSW], F32, tag="psum")

            # ---- main loop (optionally repeated on-device for timing) ----
            import contextlib

            n_iters = repeat // bodies if repeat > 1 else 1
            n_bodies = bodies if repeat > 1 else 1
            loop_cm = tc.For_i(0, n_iters, 1) if n_iters > 1 else contextlib.nullcontext()
            with loop_cm:
                # deferred-emission FIFO: consumers of freshly produced data
                # are emitted >= 1 batch (~9 us) after their producers so no
                # in-order engine queue ever stalls on a cross-engine dep.
                pending = []

                def drain(k):
                    for _ in range(min(k, len(pending))):
                        pending.pop(0)()

                for bi in range(n_bodies * n_batches):
                    bi = bi % n_batches
                    drain(max(0, len(pending) - fifo_depth))
                    t0 = bi * dma_batch
                    nb = min(dma_batch, tiles - t0)  # ragged last batch
                    mode = schedule[bi]
                    l_batch = io_pool.tile([128, dma_batch * A * C], F32, tag="l_batch")
                    r0 = t0 * ROWS_PER_TILE
                    nc.sync.dma_start(
                        out=l_batch[:, : nb * A * C].rearrange(
                            "p (b x) -> p b x", x=A * C
                        ),
                        in_=lg[r0 : r0 + nb * ROWS_PER_TILE, :].rearrange(
                            "(b p a) c -> p b a c", p=128, a=A
                        ),
                    )
                    lb = l_batch[:, : nb * A * C]
                    l4 = lb.rearrange("p (b a c) -> p b a c", a=A, c=C)

                    if "e" not in ablate:
                        e_batch = ek_pool.tile([128, dma_batch * A * C], F32, tag="e_batch")
                        eb = e_batch[:, : nb * A * C]
                        nc.scalar.activation(eb, lb, mybir.ActivationFunctionType.Exp)

                    m_slice = m_all[:, t0 * A : (t0 + nb) * A]
                    s_slice = s_all[:, t0 * A : (t0 + nb) * A]

                    if "m" not in ablate:
                        nc.vector.tensor_reduce(
                            out=m_slice, in_=l4, axis=mybir.AxisListType.X,
                            op=mybir.AluOpType.max,
                        )

                    if "s" not in ablate and "e" not in ablate:
                        e4 = eb.rearrange("p (b a c) -> p b a c", a=A, c=C)
                        if mode == "D":
                            def finish(e4=e4, s_slice=s_slice):
                                nc.vector.tensor_reduce(
                                    out=s_slice, in_=e4, axis=mybir.AxisListType.X,
                                    op=mybir.AluOpType.add,
                                )
                            pending.append(finish)
                        elif mode == "P1":
                            h1 = f1_pool.tile([128, dma_batch * A * 50], F32, tag="h1")
                            h14 = h1[:, : nb * A * 50].rearrange(
                                "p (b a c) -> p b a c", a=A, c=50
                            )
                            nc.gpsimd.tensor_tensor(
                                out=h14, in0=e4[:, :, :, 0:50], in1=e4[:, :, :, 50:100],
                                op=mybir.AluOpType.add,
                            )

                            def finish(h14=h14, s_slice=s_slice):
                                nc.vector.tensor_reduce(
                                    out=s_slice, in_=h14, axis=mybir.AxisListType.X,
                                    op=mybir.AluOpType.add,
                                )
                            pending.append(finish)
                        else:  # P2
                            h1 = f1_pool.tile([128, dma_batch * A * 50], F32, tag="h1")
                            h14 = h1[:, : nb * A * 50].rearrange(
                                "p (b a c) -> p b a c", a=A, c=50
                            )
                            nc.gpsimd.tensor_tensor(
                                out=h14, in0=e4[:, :, :, 0:50], in1=e4[:, :, :, 50:100],
                                op=mybir.AluOpType.add,
                            )
                            h2 = f2_pool.tile([128, dma_batch * A * 25], F32, tag="h2")
                            h24 = h2[:, : nb * A * 25].rearrange(
                                "p (b a c) -> p b a c", a=A, c=25
                            )
                            nc.gpsimd.tensor_tensor(
                                out=h24, in0=h14[:, :, :, 0:25], in1=h14[:, :, :, 25:50],
                                op=mybir.AluOpType.add,
                            )

                            def finish(h24=h24, s_slice=s_slice):
                                nc.vector.tensor_reduce(
                                    out=s_slice, in_=h24, axis=mybir.AxisListType.X,
                                    op=mybir.AluOpType.add,
                                )
                            pending.append(finish)

                    # ---- chunked epilogue, staged through the FIFO ----
                    if "E" not in ablate:
                        for cs, ce in chunk_bounds:
                            last = ce - 1
                            if not (t0 <= last < t0 + nb):
                                continue
                            # window-aligned col range (last chunk takes the
                            # pad cols so every window is full)
                            c0 = cs * A
                            c1 = -(-(ce * A) // WIN) * WIN  # round up to window
                            w = c1 - c0
                            w0 = c0 // WIN                     # first window
                            cw = w // WIN                      # windows in chunk
                            assert cw * WIN == w and c0 % WIN == 0
                            # tiny final chunk: all-DVE one-hot (short serial
                            # tail beats ACT's per-instruction overhead there)
                            k_chunk = N_BINS if (cs, ce) == chunk_bounds[-1] else oh_dve_bins

                            box = {}

                            def stage_a(c0=c0, c1=c1, w=w, box=box):
                                em = epi_pool.tile([128, w], F32, tag="em", name="em")
                                nc.scalar.activation(
                                    em[:], m_all[:, c0:c1], mybir.ActivationFunctionType.Exp
                                )
                                sinv = epi_pool.tile([128, w], F32, tag="sinv", name="sinv")
                                nc.vector.reciprocal(sinv[:], s_all[:, c0:c1])
                                box["a"] = (em, sinv)

                            def stage_b(c0=c0, c1=c1, w0=w0, cw=cw, box=box):
                                em, sinv = box["a"]
                                # conf/acc written strided into the lhsT windows
                                conf_dst = lhsT_w[:, w0 : w0 + cw, 1 : 1 + WIN]
                                acc_dst = lhsT_w[:, w0 : w0 + cw, 1 + WIN : LHSW]
                                nc.vector.tensor_tensor(
                                    out=conf_dst,
                                    in0=em[:].rearrange("p (w k) -> p w k", k=WIN),
                                    in1=sinv[:].rearrange("p (w k) -> p w k", k=WIN),
                                    op=mybir.AluOpType.mult,
                                )
                                nc.vector.tensor_tensor(
                                    out=acc_dst,
                                    in0=g_all[:, c0:c1].rearrange("p (w k) -> p w k", k=WIN),
                                    in1=m_all[:, c0:c1].rearrange("p (w k) -> p w k", k=WIN),
                                    op=mybir.AluOpType.is_equal,
                                )
                                box["conf"] = conf_dst

                            def stage_c(w=w, box=box):
                                conf_dst = box["conf"]
                                # ks = exact integer bin index as small f32:
                                # round(15*conf - .5) via the MAGIC add/sub
                                t = epi_pool.tile([128, w], F32, tag="bt", name="bt")
                                nc.scalar.activation(
                                    t[:].rearrange("p (w k) -> p w k", k=WIN),
                                    conf_dst,
                                    mybir.ActivationFunctionType.Copy,
                                    bias=-0.5, scale=float(N_BINS),
                                )
                                binv = epi_pool.tile([128, w], F32, tag="binv", name="binv")
                                nc.scalar.activation(
                                    binv[:], t[:], mybir.ActivationFunctionType.Copy,
                                    bias=MAGIC, scale=1.0,
                                )
                                box["binv"] = binv

                            def stage_d(w=w, k_chunk=k_chunk, box=box):
                                binv = box["binv"]
                                oh = epi_pool.tile([128, w * N_BINS], F32, tag="oh", name="oh")
                                # bin-major: each bin's one-hot write is a
                                # CONTIGUOUS [128, w] run (strided writes
                                # measured 2-4x slower on ACT/DVE)
                                if oh_binmajor:
                                    oh3 = oh[:].rearrange("p (b c) -> p b c", c=w)
                                else:
                                    oh3 = oh[:].rearrange("p (c b) -> p b c", b=N_BINS).transpose_view(1, 2) if False else oh[:].rearrange("p (c b) -> p c b", b=N_BINS)
                                u = epi_pool.tile([128, w], F32, tag="u", name="u")
                                for b in range(N_BINS):
                                    ob = oh3[:, b] if oh_binmajor else oh3[:, :, b]
                                    if b < k_chunk:
                                        nc.vector.tensor_scalar(
                                            ob, binv[:], MAGIC + float(b), None,
                                            op0=mybir.AluOpType.is_equal,
                                        )
                                    else:
                                        # u = (binv-(MAGIC+b))^2: exact small
                                        # int distance (Sterbenz); oh_b =
                                        # relu(1 - u) is an exact {0,1} mask
                                        nc.scalar.activation(
                                            u[:], binv[:],
                                            mybir.ActivationFunctionType.Square,
                                            bias=negb[:, b : b + 1], scale=1.0,
                                        )
                                        nc.scalar.activation(
                                            ob, u[:],
                                            mybir.ActivationFunctionType.Relu,
                                            bias=one_c[:, 0:1], scale=-1.0,
                                        )
                                box["oh"] = oh

                            def stage_e(cs=cs, w=w, w0=w0, cw=cw, box=box):
                                oh = box["oh"]
                                if oh_binmajor:
                                    oh3e = oh[:].rearrange("p (b c) -> p b c", c=w)
                                else:
                                    oh3e = None
                                for wi in range(cw):
                                    gw = w0 + wi
                                    first = cs == 0 and wi == 0
                                    final = cs == chunk_starts[-1] and wi == cw - 1
                                    lhsT = lhsT_all[:, gw * LHSW : (gw + 1) * LHSW]
                                    if oh_binmajor:
                                        rhs = oh3e[:, :, wi * WIN : (wi + 1) * WIN]
                                    else:
                                        rhs = oh[:, wi * WIN * N_BINS : (wi + 1) * WIN * N_BINS]
                                    nc.tensor.matmul(
                                        out=psum[:, :],
                                        lhsT=lhsT,
                                        rhs=rhs,
                                        start=first,
                                        stop=final,
                                    )

                            pending.extend([stage_a, stage_b, stage_c, stage_d, stage_e])

                drain(len(pending))

            if "E" not in ablate:
                out_sb = stage.tile([LHSW, RHSW], F32, tag="out_sb")
                nc.vector.tensor_copy(out_sb[:], psum[:])
                nc.sync.dma_start(out=part[:], in_=out_sb[:])
            if dbg is not None:
                for i, src in enumerate((m_all, s_all, g_all)):
                    nc.sync.dma_start(out=dbg[:, i * cols : (i + 1) * cols], in_=src[:])

    return nc


# ----------------------------------------------------------------------------
# host side
# ----------------------------------------------------------------------------

_RUNNER_CACHE = {}


def _get_runner(tiles=TILES, repeat=1, ncores=N_CORES, **opts):
    """Build (once) a jitted shard_map runner for the kernel."""
    key = (tiles, repeat, ncores, tuple(sorted((k, str(v)) for k, v in opts.items())))
    if key in _RUNNER_CACHE:
        return _RUNNER_CACHE[key]

    import jax
    from jax.sharding import Mesh, PartitionSpec
    try:
        from jax.experimental.shard_map import shard_map
    except ImportError:
        from jax.shard_map import shard_map
    from concourse import bass2jax

    nc = build_nc(tiles, repeat=repeat, **opts)
    bass2jax.install_neuronx_cc_hook()

    partition_name = nc.partition_id_tensor.name if nc.partition_id_tensor else None
    in_names = ["lg", "gg"]
    out_names = ["part"]
    out_avals = [jax.core.ShapedArray((LHSW, RHSW), np.float32)]
    if opts.get("debug_out"):
        out_names.append("dbg")
        out_avals.append(jax.core.ShapedArray((128, tiles * A * 4), np.float32))
    all_in_names = in_names + out_names + ([partition_name] if partition_name else [])

    def _body(*args):
        operands = list(args)
        if partition_name is not None:
            operands.append(bass2jax.partition_id_tensor())
        outs = bass2jax._bass_exec_p.bind(
            *operands,
            out_avals=tuple(out_avals),
            in_names=tuple(all_in_names),
            out_names=tuple(out_names),
            lowering_input_output_aliases=(),
            sim_require_finite=True,
            sim_require_nnan=True,
            nc=nc,
        )
        return tuple(outs)

    devices = jax.devices()[:ncores]
    mesh = Mesh(np.asarray(devices), ("core",))
    n_in = len(in_names) + len(out_avals)
    sharded = jax.jit(
        shard_map(
            _body,
            mesh=mesh,
            in_specs=(PartitionSpec("core"),) * n_in,
            out_specs=(PartitionSpec("core"),) * len(out_names),
            check_rep=False,
        ),
        donate_argnums=(len(in_names),),
        keep_unused=True,
    )
    _RUNNER_CACHE[key] = sharded
    return sharded


def _prep_inputs(logits, labels):
    logits = np.asarray(logits)
    labels = np.asarray(labels).astype(np.int64)
    n = logits.shape[0]
    assert logits.shape == (N_FULL, C) and n == N_FULL, logits.shape
    pad = N_PAD - n
    lg = np.concatenate([logits, np.zeros((pad, C), np.float32)], axis=0)
    # label-logit gather; pad rows get -1 (pad m = 0 -> acc = 0, bin 0)
    g = np.empty(N_PAD, np.float32)
    g[:n] = logits[np.arange(n), labels]
    g[n:] = -1.0
    # arrange to the device layout: gg[core][p, t*A + a] = g[core, t, p, a]
    gg = np.ascontiguousarray(
        g.reshape(N_CORES, TILES, 128, A).transpose(0, 2, 1, 3)
    ).reshape(N_CORES * 128, TILES * A)
    return lg, gg, pad


def _finish(parts, pad):
    """parts: (8, 65, 480) psum dumps -> ECE scalar (f32 [1]).

    psum col j*15+b holds, for window col-slot j: count in row 0, conf_sum
    in row 1+j, acc_sum in row 33+j (other rows unused cross terms)."""
    v = parts.reshape(parts.shape[0], LHSW, N_BINS, WIN).astype(np.float64)
    j = np.arange(WIN)
    counts = v[:, 0, :, :].sum(axis=(0, 2))
    conf_sums = v[:, 1 + j, :, j].sum(axis=(0, 1))
    acc_sums = v[:, 1 + WIN + j, :, j].sum(axis=(0, 1))
    # all-zero pad rows: conf = exp(0)/100 -> bin 0, acc = 0 (g=-1 != m=0)
    counts[0] -= pad
    conf_sums[0] -= pad * float(np.float32(1.0) / np.float32(100.0))
    n = N_FULL
    prop = counts / n
    denom = np.maximum(counts, 1.0)
    avg_conf = conf_sums / denom
    avg_acc = acc_sums / denom
    per_bin = np.where(counts > 0, np.abs(avg_conf - avg_acc) * prop, 0.0)
    return np.array([per_bin.sum()], dtype=np.float32)


def kernel(logits, labels):
    lg, gg, pad = _prep_inputs(logits, labels)
    runner = _get_runner()
    zeros = np.zeros((N_CORES * LHSW, RHSW), np.float32)
    last = None
    for attempt in range(3):
        try:
            (out,) = runner(lg, gg, np.zeros_like(zeros))
            parts = np.asarray(out).reshape(N_CORES, LHSW, RHSW)
            return _finish(parts, pad)
        except Exception as e:  # transient NRT_EXEC_UNIT_UNRECOVERABLE etc.
            last = e
            import time as _time

            _time.sleep(20)
    raise last
